# revision 18
# baseline (speedup 1.0000x reference)
"""MoE AlltoAllTokenDispatcher kernel for TRN2 (8 NeuronCores).

The reference dispatcher's gather (tokens[argsort(idx)//k]) followed by
scatter-add at the same argsort permutation is an exact identity on slot
order: unpermuted[s] == tokens[s // k] for every slot s, independent of the
routing indices. The whole module therefore reduces to

    out[i] = tokens[i] * (probs[i, 0] + probs[i, 1])

a pure memory-bound row-scaling (read 256 MB + write 256 MB). Tokens are
sharded across the 8 cores on the token dim (data-parallel per the sharding
hint; no all-to-all is needed since the expert compute between dispatch and
combine is identity).

Per-core kernel (Tile framework):
  - Tile i, partition p <-> token row 16p + i. With that tiling probs
    loads as ONE fully contiguous [128, 32] tile and a single strided DVE
    pair-add produces every tile's per-partition scale column.
  - HWDGE lane rule (measured by probe): a [P, C] DMA splits across the
    largest divisor of P that is <= 16 SDMA engines, lanes always
    starting at engine 0 (124 -> 4 lanes/engines, ~104 GB/s disaster;
    120 -> 15 lanes, engine 15 idle; multiples of 16 -> all 16).
  - SDMA engine 15 sporadically runs ~13% slower per byte (observed in
    1 of 2 baseline traces at ~99% duty, pacing the whole window). Tiles
    13/14 therefore use 120 partitions (engine 15 idle), shifting ~12.5%
    of bytes off engine 15: in slow-engine-15 draws the window drops to
    the HBM bound (~-14 us), in healthy draws it costs <~1.5 us. The 16
    leftover rows ride two tiny [8, 4096] chunks (8 lanes each).
  - Loads ride the sync HWDGE ring; stores AND the probs loads ride the
    scalar HWDGE ring (idle early), so the sync ring's first dispatch is
    a token-tile load and the window starts ~1 us earlier.
  - First tile ramps 1024/1024/2048 cols (first store chain starts
    early); the last tile tapers 2048/1024/512/256/128/64/64 cols so the
    final load->mul->store chain after the last load byte is ~2 us (the
    v1 drain spent 9.4 us trickling at 128 GB/s).
"""

import numpy as np

import concourse.tile as tile
from concourse import bacc, mybir
from concourse.bass_utils import run_bass_kernel_spmd

N_TOKENS = 16384
HIDDEN = 4096
TOP_K = 2
N_CORES = 8
TOK_PER_CORE = N_TOKENS // N_CORES  # 2048
P = 128
N_TILES = TOK_PER_CORE // P  # 16
N_BUFS = 8

_nc_cache = None

_RAMP = (1024, 1024, 2048)  # tile 0: small first piece -> early first store
_TAPER = (2048, 1024, 512, 256, 128, 64, 64)  # tile 15: short final chain
P120_TILES = (13, 14)  # these tiles use partitions [0:120) -> 15 DMA lanes


def _work_items():
    """(tile_idx, col_start, ncols): first tile ramped, last tile tapered.
    "M" is the 16-row leftover chunk of the two 120-partition tiles."""
    items = []
    c = 0
    for w in _RAMP:
        items.append((0, c, w))
        c += w
    assert c == HIDDEN
    items.append(("M", 0, HIDDEN))
    for i in range(1, N_TILES - 1):
        items.append((i, 0, HIDDEN))
    c = 0
    for w in _TAPER:
        items.append((N_TILES - 1, c, w))
        c += w
    assert c == HIDDEN
    return items


def _build_nc():
    nc = bacc.Bacc(
        "TRN2", target_bir_lowering=False, debug=False, num_devices=N_CORES
    )
    tokens = nc.dram_tensor(
        "tokens", [TOK_PER_CORE, HIDDEN], mybir.dt.float32, kind="ExternalInput"
    ).ap()
    probs = nc.dram_tensor(
        "probs", [TOK_PER_CORE, TOP_K], mybir.dt.float32, kind="ExternalInput"
    ).ap()
    out = nc.dram_tensor(
        "out", [TOK_PER_CORE, HIDDEN], mybir.dt.float32, kind="ExternalOutput"
    ).ap()
    # tile i, partition p  <->  token row 16p + i
    tok_t = tokens.rearrange("(p n) m -> n p m", n=N_TILES)
    out_t = out.rearrange("(p n) m -> n p m", n=N_TILES)

    with tile.TileContext(nc) as tc:
        with (
            tc.tile_pool(name="tok", bufs=N_BUFS) as tok_pool,
            tc.tile_pool(name="pr", bufs=1) as pr_pool,
        ):
            # pt[p, (j k)] <- probs[16p+j, k]: one contiguous DMA, then
            # st[p, j] = pt[p, 2j] + pt[p, 2j+1]: one strided DVE add.
            pt = pr_pool.tile([P, N_TILES * TOP_K], mybir.dt.float32, tag="pt")
            st = pr_pool.tile([P, N_TILES], mybir.dt.float32, tag="st")
            ptM = pr_pool.tile([P, TOP_K], mybir.dt.float32, tag="ptM")
            stM = pr_pool.tile([P, 1], mybir.dt.float32, tag="stM")
            nc.scalar.dma_start(
                out=pt[:],
                in_=probs.rearrange("(p j) k -> p (j k)", j=N_TILES),
            )
            # leftover rows 16p+j for p in [120,128), j in P120_TILES:
            # mini-chunk partitions [0:8) <- tile j0 rows, [8:16) <- tile j1
            # rows (two tiny [8, 2] strided loads).
            probs_t = probs.rearrange("(p j) k -> j p k", j=N_TILES)
            j0, j1 = P120_TILES
            nc.scalar.dma_start(out=ptM[0:8, :], in_=probs_t[j0, 120:128, :])
            nc.scalar.dma_start(out=ptM[8:16, :], in_=probs_t[j1, 120:128, :])
            pt3 = pt[:].rearrange("p (j k) -> p j k", k=TOP_K)
            nc.vector.tensor_add(
                st[:].rearrange("p (j o) -> p j o", o=1),
                pt3[:, :, 0:1],
                pt3[:, :, 1:2],
            )
            nc.vector.tensor_add(stM[0:16, :], ptM[0:16, 0:1], ptM[0:16, 1:2])

            for i, c0, ncols in _work_items():
                tt = tok_pool.tile([P, HIDDEN], mybir.dt.float32, tag="tok")
                if i == "M":
                    nc.sync.dma_start(
                        out=tt[0:8, :ncols],
                        in_=tok_t[j0, 120:128, c0 : c0 + ncols],
                    )
                    nc.sync.dma_start(
                        out=tt[8:16, :ncols],
                        in_=tok_t[j1, 120:128, c0 : c0 + ncols],
                    )
                    nc.vector.tensor_scalar_mul(
                        tt[0:16, :ncols], tt[0:16, :ncols], stM[0:16, 0:1]
                    )
                    nc.scalar.dma_start(
                        out=out_t[j0, 120:128, c0 : c0 + ncols],
                        in_=tt[0:8, :ncols],
                    )
                    nc.scalar.dma_start(
                        out=out_t[j1, 120:128, c0 : c0 + ncols],
                        in_=tt[8:16, :ncols],
                    )
                    continue
                pp = 120 if i in P120_TILES else P
                nc.sync.dma_start(
                    out=tt[0:pp, :ncols], in_=tok_t[i, 0:pp, c0 : c0 + ncols]
                )
                nc.vector.tensor_scalar_mul(
                    tt[0:pp, :ncols], tt[0:pp, :ncols], st[0:pp, i : i + 1]
                )
                nc.scalar.dma_start(
                    out=out_t[i, 0:pp, c0 : c0 + ncols], in_=tt[0:pp, :ncols]
                )
    nc.compile()
    return nc


def kernel(tokens, probs, indices=None, **_unused):
    global _nc_cache
    tokens = np.ascontiguousarray(np.asarray(tokens, dtype=np.float32))
    probs = np.ascontiguousarray(np.asarray(probs, dtype=np.float32))
    assert tokens.shape == (N_TOKENS, HIDDEN), tokens.shape
    assert probs.shape == (N_TOKENS, TOP_K), probs.shape

    if _nc_cache is None:
        _nc_cache = _build_nc()

    in_maps = [
        {
            "tokens": tokens[c * TOK_PER_CORE : (c + 1) * TOK_PER_CORE],
            "probs": probs[c * TOK_PER_CORE : (c + 1) * TOK_PER_CORE],
        }
        for c in range(N_CORES)
    ]
    res = run_bass_kernel_spmd(
        _nc_cache, in_maps, core_ids=list(range(N_CORES))
    )
    return np.concatenate([res.results[c]["out"] for c in range(N_CORES)], axis=0)


# revision 21
# speedup vs baseline: 1.2146x; 1.2146x over previous
"""MoE AlltoAllTokenDispatcher kernel for TRN2 (8 NeuronCores).

The reference dispatcher's gather (tokens[argsort(idx)//k]) followed by
scatter-add at the same argsort permutation is an exact identity on slot
order: unpermuted[s] == tokens[s // k] for every slot s, independent of the
routing indices. The whole module therefore reduces to

    out[i] = tokens[i] * (probs[i, 0] + probs[i, 1])

a pure memory-bound row-scaling (read 256 MB + write 256 MB). Tokens are
sharded across the 8 cores on the token dim (data-parallel per the sharding
hint; no all-to-all is needed since the expert compute between dispatch and
combine is identity).

Per-core kernel (Tile framework):
  - Tile i, partition p <-> token row 16p + i. With that tiling probs
    loads as ONE fully contiguous [128, 32] tile and a single strided DVE
    pair-add produces every tile's per-partition scale column.
  - HWDGE lane rule (measured by probe): a [P, C] DMA splits across the
    largest divisor of P that is <= 16 SDMA engines, lanes always
    starting at engine 0 (124 -> 4 lanes/engines, ~104 GB/s disaster;
    120 -> 15 lanes, engine 15 idle; multiples of 16 -> all 16).
  - SDMA engine 15 sporadically runs ~13% slower per byte (observed in
    1 of 7 traces at ~99% duty, pacing the whole window). Mid-kernel
    tiles 6/7 therefore use 120 partitions (engine 15 idle), shifting
    ~12.5% of bytes off engine 15: in slow-engine-15 draws the window
    drops to the HBM bound (~-14 us), in healthy draws it costs
    <~1.5 us. The 16 leftover rows ride two tiny [8, 4096] chunks
    (8 lanes each); keeping the 15-lane tiles mid-kernel leaves the
    drain full-width.
  - Loads ride the sync HWDGE ring; stores AND the probs loads ride the
    scalar HWDGE ring (idle early), so the sync ring's first dispatch is
    a token-tile load and the window starts ~1 us earlier.
  - First tile ramps 1024/1024/2048 cols (first store chain starts
    early); tiles 13/14/15 taper down to 64-col pieces so the store
    backlog at load-end is small and the final load->mul->store chain
    after the last load byte is ~2 us (the v1 drain spent 9.4 us
    trickling at 128 GB/s).
"""

import numpy as np

import concourse.tile as tile
from concourse import bacc, mybir
from concourse.bass_utils import run_bass_kernel_spmd

N_TOKENS = 16384
HIDDEN = 4096
TOP_K = 2
N_CORES = 8
TOK_PER_CORE = N_TOKENS // N_CORES  # 2048
P = 128
N_TILES = TOK_PER_CORE // P  # 16
N_BUFS = 8

_nc_cache = None

_RAMP = (1024, 1024, 2048)  # tile 0: small first piece -> early first store
# Tail tiles split so the store backlog left at load-end is small and the
# final load->mul->store chain is short; pieces stay >=64 cols (256B descs).
_SPLITS = {
    13: (2048, 2048),
    14: (2048, 1024, 1024),
    15: (2048, 1024, 512, 256, 128, 64, 64),
}
P120_TILES = (6, 7)  # mid-kernel tiles on partitions [0:120) -> 15 DMA lanes


def _work_items():
    """(tile_idx, col_start, ncols): first tile ramped, tail tiles tapered.
    "M" is the 16-row leftover chunk of the two 120-partition tiles."""
    items = []
    c = 0
    for w in _RAMP:
        items.append((0, c, w))
        c += w
    assert c == HIDDEN
    items.append(("M", 0, HIDDEN))
    for i in range(1, N_TILES):
        c = 0
        for w in _SPLITS.get(i, (HIDDEN,)):
            items.append((i, c, w))
            c += w
        assert c == HIDDEN
    return items


def _build_nc():
    nc = bacc.Bacc(
        "TRN2", target_bir_lowering=False, debug=False, num_devices=N_CORES
    )
    tokens = nc.dram_tensor(
        "tokens", [TOK_PER_CORE, HIDDEN], mybir.dt.float32, kind="ExternalInput"
    ).ap()
    probs = nc.dram_tensor(
        "probs", [TOK_PER_CORE, TOP_K], mybir.dt.float32, kind="ExternalInput"
    ).ap()
    out = nc.dram_tensor(
        "out", [TOK_PER_CORE, HIDDEN], mybir.dt.float32, kind="ExternalOutput"
    ).ap()
    # tile i, partition p  <->  token row 16p + i
    tok_t = tokens.rearrange("(p n) m -> n p m", n=N_TILES)
    out_t = out.rearrange("(p n) m -> n p m", n=N_TILES)

    with tile.TileContext(nc) as tc:
        with (
            tc.tile_pool(name="tok", bufs=N_BUFS) as tok_pool,
            tc.tile_pool(name="pr", bufs=1) as pr_pool,
        ):
            # pt[p, (j k)] <- probs[16p+j, k]: one contiguous DMA, then
            # st[p, j] = pt[p, 2j] + pt[p, 2j+1]: one strided DVE add.
            pt = pr_pool.tile([P, N_TILES * TOP_K], mybir.dt.float32, tag="pt")
            st = pr_pool.tile([P, N_TILES], mybir.dt.float32, tag="st")
            ptM = pr_pool.tile([P, TOP_K], mybir.dt.float32, tag="ptM")
            stM = pr_pool.tile([P, 1], mybir.dt.float32, tag="stM")
            nc.scalar.dma_start(
                out=pt[:],
                in_=probs.rearrange("(p j) k -> p (j k)", j=N_TILES),
            )
            # leftover rows 16p+j for p in [120,128), j in P120_TILES:
            # mini-chunk partitions [0:8) <- tile j0 rows, [8:16) <- tile j1
            # rows (two tiny [8, 2] strided loads).
            probs_t = probs.rearrange("(p j) k -> j p k", j=N_TILES)
            j0, j1 = P120_TILES
            nc.scalar.dma_start(out=ptM[0:8, :], in_=probs_t[j0, 120:128, :])
            nc.scalar.dma_start(out=ptM[8:16, :], in_=probs_t[j1, 120:128, :])
            pt3 = pt[:].rearrange("p (j k) -> p j k", k=TOP_K)
            nc.vector.tensor_add(
                st[:].rearrange("p (j o) -> p j o", o=1),
                pt3[:, :, 0:1],
                pt3[:, :, 1:2],
            )
            nc.vector.tensor_add(stM[0:16, :], ptM[0:16, 0:1], ptM[0:16, 1:2])

            for i, c0, ncols in _work_items():
                tt = tok_pool.tile([P, HIDDEN], mybir.dt.float32, tag="tok")
                if i == "M":
                    nc.sync.dma_start(
                        out=tt[0:8, :ncols],
                        in_=tok_t[j0, 120:128, c0 : c0 + ncols],
                    )
                    nc.sync.dma_start(
                        out=tt[8:16, :ncols],
                        in_=tok_t[j1, 120:128, c0 : c0 + ncols],
                    )
                    nc.vector.tensor_scalar_mul(
                        tt[0:16, :ncols], tt[0:16, :ncols], stM[0:16, 0:1]
                    )
                    nc.scalar.dma_start(
                        out=out_t[j0, 120:128, c0 : c0 + ncols],
                        in_=tt[0:8, :ncols],
                    )
                    nc.scalar.dma_start(
                        out=out_t[j1, 120:128, c0 : c0 + ncols],
                        in_=tt[8:16, :ncols],
                    )
                    continue
                pp = 120 if i in P120_TILES else P
                nc.sync.dma_start(
                    out=tt[0:pp, :ncols], in_=tok_t[i, 0:pp, c0 : c0 + ncols]
                )
                nc.vector.tensor_scalar_mul(
                    tt[0:pp, :ncols], tt[0:pp, :ncols], st[0:pp, i : i + 1]
                )
                nc.scalar.dma_start(
                    out=out_t[i, 0:pp, c0 : c0 + ncols], in_=tt[0:pp, :ncols]
                )
    nc.compile()
    return nc


def kernel(tokens, probs, indices=None, **_unused):
    global _nc_cache
    tokens = np.ascontiguousarray(np.asarray(tokens, dtype=np.float32))
    probs = np.ascontiguousarray(np.asarray(probs, dtype=np.float32))
    assert tokens.shape == (N_TOKENS, HIDDEN), tokens.shape
    assert probs.shape == (N_TOKENS, TOP_K), probs.shape

    if _nc_cache is None:
        _nc_cache = _build_nc()

    in_maps = [
        {
            "tokens": tokens[c * TOK_PER_CORE : (c + 1) * TOK_PER_CORE],
            "probs": probs[c * TOK_PER_CORE : (c + 1) * TOK_PER_CORE],
        }
        for c in range(N_CORES)
    ]
    res = run_bass_kernel_spmd(
        _nc_cache, in_maps, core_ids=list(range(N_CORES))
    )
    return np.concatenate([res.results[c]["out"] for c in range(N_CORES)], axis=0)
